# revision 6
# baseline (speedup 1.0000x reference)
"""BlockLinear kernel for Trainium2 (8 NeuronCores, SPMD).

y[b, g*512+o] = sum_i x[b, g*512+i] * W[g, o, i] + bias[g, o]

Sharding: one block g per core (expert parallelism). Each core computes
y_g = x_g @ W_g^T + b_g with x_g = x[:, g*512:(g+1)*512].

Per-core device kernel:
  - inputs: xT [512, 16384] (x_g transposed + cast on host), wT [512, 512]
    fp16 (W_g^T = [in, out]), bias [128, 512] fp32 (replicated over
    partitions)
  - x-in DMAs run alone on the sync HWDGE ring; W (split per k-tile so the
    first matmul only waits on 128 KB), bias, and all y-out DMAs run on the
    scalar HWDGE ring; the last two tail groups' y DMAs go back on the
    sync ring (idle by then).  Batch is processed in groups (geometric
    head ramp, 1536-row body, 512/256/128/128 tail so the final output
    DMA is small).  Per 128-row subtile: 4 accumulating matmuls into a
    PSUM bank, DVE adds bias while copying PSUM->SBUF (casting to fp16).
  - warmup fp32 matmuls on a small memset tile bridge the initial DMA
    fill so the PE HAM clock-gate is ramping while x/W land.

Schemes: "f16" (x,W fp16; rel err ~3e-4), "f8x" (x fp8-e3m4, W fp16 —
halves x DMA bytes; PE rate identical; rel err ~1e-2).
"""

import numpy as np

import concourse.bass as bass
import concourse.mybir as mybir
import concourse.tile as tile
from concourse import bacc
from concourse.bass_utils import run_bass_kernel_spmd
from concourse.vector_clock import ScopedClock

F32 = mybir.dt.float32

NB, BIN, BOUT = 8, 512, 512
BATCH = 16384
NCORES = 8
P = 128
KT = BIN // P  # 4 k-tiles per block

SCHEME = "f8x"  # "f16" | "f8x"

_patched = False


def _patch_tile_drain():
    """Walrus in this container accepts only one sync-wait per InstDrain;
    split the tile-exit drain's waits across one drain instruction each."""
    global _patched
    if _patched:
        return
    _patched = True

    def _drain_and_barrier(self, tick_clock, wait_clock):
        nc = self.nc
        drain_inst = nc.sync.drain()
        wait_clock.add_sem_waits(
            drain_inst.ins, ScopedClock({None: tick_clock.global_clock})
        )
        si = drain_inst.ins.sync_info
        if si is not None and len(si.on_wait) > 1:
            waits = list(si.on_wait)
            updates = list(si.on_update)
            drain_inst.ins.sync_info = mybir.SyncInfo(
                on_wait=[waits[0]], on_update=updates
            )
            for w in waits[1:]:
                extra = nc.sync.drain()
                extra.ins.sync_info = mybir.SyncInfo(on_wait=[w], on_update=[])
        nc.all_engine_barrier()
        popped = nc._tile_sem_poison_stack.pop()
        assert popped is self._sem_poison
        # Skip Tile's exit-time sem clear + second barrier: walrus's
        # end-of-NEFF epilogue unconditionally zeroes every semaphore on
        # every engine, and nothing runs between the barrier above and
        # that epilogue. (Verified: repeated executions stay correct.)
        sems = list(self.sems.allocated().values())
        sem_nums = [s.num if hasattr(s, "num") else s for s in sems]
        nc._state.prepend_free_semaphores(sem_nums)
        for poison_set in nc._tile_sem_poison_stack:
            poison_set.update(sem_nums)

    tile.TileContext._drain_and_barrier = _drain_and_barrier


_nc_cache = {}


def _scheme_dtypes(scheme):
    if scheme == "f16":
        return mybir.dt.float16, np.float16
    elif scheme == "f8x":
        import ml_dtypes

        return mybir.dt.float8e3, ml_dtypes.float8_e3m4
    raise ValueError(scheme)


def _groups(body=1536):
    """Batch-row group sizes: geometric head ramp (matmuls start on the
    first small tile while DMA builds runway), fixed-size body, small
    final groups (the kernel tail only waits on small output DMAs)."""
    head = [256, 512, 1024]
    tail = [512, 256, 128, 128]
    mid = BATCH - sum(head) - sum(tail)
    sizes = head + [body] * (mid // body)
    rem = mid % body
    if rem:
        sizes.append(rem)
    sizes += tail
    assert sum(sizes) == BATCH, sizes
    return sizes


def _build(scheme=SCHEME, body=1536):
    key = (scheme, body)
    if key in _nc_cache:
        return _nc_cache[key]
    _patch_tile_drain()
    x_dt, _ = _scheme_dtypes(scheme)
    w_dt = mybir.dt.float16
    out_dt = mybir.dt.float16

    nc = bacc.Bacc(None, target_bir_lowering=False)
    xT = nc.dram_tensor("xT", [BIN, BATCH], x_dt, kind="ExternalInput")
    wT = nc.dram_tensor("wT", [BIN, BOUT], w_dt, kind="ExternalInput")
    bias = nc.dram_tensor("bias", [P, BOUT], F32, kind="ExternalInput")
    y = nc.dram_tensor("y", [BATCH, BOUT], out_dt, kind="ExternalOutput")

    groups = _groups(body)

    with tile.TileContext(nc) as tc:
        with (
            tc.tile_pool(name="const", bufs=1) as const,
            tc.tile_pool(name="xp", bufs=10 if scheme == "f8x" else 6) as xp,
            tc.tile_pool(name="yp", bufs=4) as yp,
            tc.tile_pool(name="ps", bufs=8, space="PSUM") as psp,
        ):
            # ---- emission order per engine is execution order ----
            # sync:   x DMAs (group 0 split per k-tile), tail y DMAs
            # scalar: W per-k DMAs, bias DMA, per-group y DMAs
            # tensor: warmup matmuls, then the real matmul stream
            # vector: per-subtile bias-add + PSUM->SBUF copy
            # gpsimd: one small memset for the warmup operands

            g0 = groups[0]
            x0k = []
            for k in range(KT):
                t = const.tile([P, g0], x_dt)
                nc.sync.dma_start(t[:], xT[k * P : (k + 1) * P, :g0])
                x0k.append(t)

            wk = const.tile([P, KT, BOUT], w_dt)
            for k in range(KT):
                nc.scalar.dma_start(wk[:, k, :], wT[k * P : (k + 1) * P, :])
            bt = const.tile([P, BOUT], F32)
            nc.scalar.dma_start(bt[:], bias[:])

            # PE warmup: fp16 matmuls with a cheap memset dependency keep
            # the PE busy (and the HAM clock-gate ramping) while the first
            # x/W tiles are still in flight; 427ns each cold gives a fine
            # handoff granularity to the first real matmul.
            junk = const.tile([P, 640], mybir.dt.float16)
            nc.gpsimd.memset(junk[:], 0.0)
            warm_ps = psp.tile([P, BOUT], F32, tag="ps")
            for _ in range(6):
                nc.tensor.matmul(
                    warm_ps[:, :],
                    junk[:, :128],
                    junk[:, 128:640],
                    start=True,
                    stop=True,
                )

            row = 0
            for gi, gsz in enumerate(groups):
                nsub = gsz // P
                if gi == 0:
                    xs = None
                else:
                    xs = xp.tile([P, KT, gsz], x_dt, tag="xt")
                    nc.sync.dma_start(
                        xs[:],
                        xT[:, row : row + gsz].rearrange("(t p) b -> p t b", p=P),
                    )
                yt = yp.tile([P, nsub, BOUT], out_dt, tag="yt")
                for ms in range(nsub):
                    ps = psp.tile([P, BOUT], F32, tag="ps")
                    for k in range(KT):
                        lhsT = (
                            x0k[k][:, ms * P : (ms + 1) * P]
                            if gi == 0
                            else xs[:, k, ms * P : (ms + 1) * P]
                        )
                        nc.tensor.matmul(
                            ps[:],
                            lhsT,
                            wk[:, k, :],
                            start=(k == 0),
                            stop=(k == KT - 1),
                        )
                    nc.vector.tensor_add(out=yt[:, ms, :], in0=ps[:], in1=bt[:])
                # tail y DMAs ride the sync ring, idle once x-in is done
                eng = nc.sync if gi >= len(groups) - 2 else nc.scalar
                eng.dma_start(
                    y[row : row + gsz, :].rearrange("(s p) o -> p s o", p=P),
                    yt[:],
                )
                row += gsz
    nc.compile()
    _nc_cache[key] = nc
    return nc


LAST_RESULT = None


def kernel(x, W, b, trace=False, scheme=SCHEME, body=1536, trace_kwargs=None):
    global LAST_RESULT
    x = np.asarray(x, dtype=np.float32)
    W = np.asarray(W, dtype=np.float32)
    b = np.asarray(b, dtype=np.float32)

    _, x_np = _scheme_dtypes(scheme)
    nc = _build(scheme, body)
    in_maps = []
    for g in range(NCORES):
        xT_g = np.ascontiguousarray(x[:, g * BIN : (g + 1) * BIN].T.astype(x_np))
        wT_g = np.ascontiguousarray(W[g].T.astype(np.float16))
        bias_g = np.ascontiguousarray(np.broadcast_to(b[g][None, :], (P, BOUT)))
        in_maps.append({"xT": xT_g, "wT": wT_g, "bias": bias_g})

    kwargs = dict(trace_kwargs or {})
    res = run_bass_kernel_spmd(nc, in_maps, list(range(NCORES)), trace=trace, **kwargs)
    LAST_RESULT = res

    out = np.empty((BATCH, NB * BOUT), dtype=np.float32)
    for g in range(NCORES):
        out[:, g * BOUT : (g + 1) * BOUT] = res.results[g]["y"].astype(np.float32)
    return out


# revision 8
# speedup vs baseline: 1.0333x; 1.0333x over previous
"""BlockLinear kernel for Trainium2 (8 NeuronCores, SPMD).

y[b, g*512+o] = sum_i x[b, g*512+i] * W[g, o, i] + bias[g, o]

Sharding: one block g per core (expert parallelism). Each core computes
y_g = x_g @ W_g^T + b_g with x_g = x[:, g*512:(g+1)*512].

Per-core device kernel:
  - inputs: xT [512, 16384] (x_g transposed + cast on host), wT [512, 512]
    fp16 (W_g^T = [in, out]), bias [128, 512] fp32 (replicated over
    partitions)
  - x-in DMAs run alone on the sync HWDGE ring; W (split per k-tile so the
    first matmul only waits on 128 KB), bias, and all y-out DMAs run on the
    scalar HWDGE ring; the last two tail groups' y DMAs go back on the
    sync ring (idle by then).  Batch is processed in groups (geometric
    head ramp, 1536-row body, 512/256/128/128 tail so the final output
    DMA is small).  Per 128-row subtile: 4 accumulating matmuls into a
    PSUM bank, DVE adds bias while copying PSUM->SBUF (casting to fp16).
  - warmup fp32 matmuls on a small memset tile bridge the initial DMA
    fill so the PE HAM clock-gate is ramping while x/W land.

Schemes: "f16" (x,W fp16; rel err ~3e-4), "f8x" (x fp8-e3m4, W fp16 —
halves x DMA bytes; PE rate identical; rel err ~1e-2).
"""

import numpy as np

import concourse.bass as bass
import concourse.mybir as mybir
import concourse.tile as tile
from concourse import bacc
from concourse.bass_utils import run_bass_kernel_spmd
from concourse.vector_clock import ScopedClock

F32 = mybir.dt.float32

NB, BIN, BOUT = 8, 512, 512
BATCH = 16384
NCORES = 8
P = 128
KT = BIN // P  # 4 k-tiles per block

SCHEME = "f8x"  # "f16" | "f8x"

_patched = False


def _patch_tile_drain():
    """Walrus in this container accepts only one sync-wait per InstDrain;
    split the tile-exit drain's waits across one drain instruction each."""
    global _patched
    if _patched:
        return
    _patched = True

    def _drain_and_barrier(self, tick_clock, wait_clock):
        nc = self.nc
        drain_inst = nc.sync.drain()
        wait_clock.add_sem_waits(
            drain_inst.ins, ScopedClock({None: tick_clock.global_clock})
        )
        si = drain_inst.ins.sync_info
        if si is not None and len(si.on_wait) > 1:
            waits = list(si.on_wait)
            updates = list(si.on_update)
            drain_inst.ins.sync_info = mybir.SyncInfo(
                on_wait=[waits[0]], on_update=updates
            )
            for w in waits[1:]:
                extra = nc.sync.drain()
                extra.ins.sync_info = mybir.SyncInfo(on_wait=[w], on_update=[])
        nc.all_engine_barrier()
        popped = nc._tile_sem_poison_stack.pop()
        assert popped is self._sem_poison
        # Skip Tile's exit-time sem clear + second barrier: walrus's
        # end-of-NEFF epilogue unconditionally zeroes every semaphore on
        # every engine, and nothing runs between the barrier above and
        # that epilogue. (Verified: repeated executions stay correct.)
        sems = list(self.sems.allocated().values())
        sem_nums = [s.num if hasattr(s, "num") else s for s in sems]
        nc._state.prepend_free_semaphores(sem_nums)
        for poison_set in nc._tile_sem_poison_stack:
            poison_set.update(sem_nums)

    tile.TileContext._drain_and_barrier = _drain_and_barrier


_nc_cache = {}


def _scheme_dtypes(scheme):
    if scheme == "f16":
        return mybir.dt.float16, np.float16
    elif scheme == "f8x":
        import ml_dtypes

        return mybir.dt.float8e3, ml_dtypes.float8_e3m4
    raise ValueError(scheme)


def _groups(body=1536):
    """Batch-row group sizes: geometric head ramp (matmuls start on the
    first small tile while DMA builds runway), fixed-size body, small
    final groups (the kernel tail only waits on small output DMAs)."""
    head = [256, 512, 1024]
    tail = [512, 256, 128, 128]
    mid = BATCH - sum(head) - sum(tail)
    sizes = head + [body] * (mid // body)
    rem = mid % body
    if rem:
        sizes.append(rem)
    sizes += tail
    assert sum(sizes) == BATCH, sizes
    return sizes


def _build(scheme=SCHEME, body=1536):
    key = (scheme, body)
    if key in _nc_cache:
        return _nc_cache[key]
    _patch_tile_drain()
    x_dt, _ = _scheme_dtypes(scheme)
    w_dt = mybir.dt.float16
    out_dt = mybir.dt.float16

    nc = bacc.Bacc(None, target_bir_lowering=False)
    xT = nc.dram_tensor("xT", [BIN, BATCH], x_dt, kind="ExternalInput")
    wT = nc.dram_tensor("wT", [BIN, BOUT], w_dt, kind="ExternalInput")
    bias = nc.dram_tensor("bias", [P, BOUT], F32, kind="ExternalInput")
    y = nc.dram_tensor("y", [BATCH, BOUT], out_dt, kind="ExternalOutput")

    groups = _groups(body)

    with tile.TileContext(nc) as tc:
        with (
            tc.tile_pool(name="const", bufs=1) as const,
            tc.tile_pool(name="xp", bufs=10 if scheme == "f8x" else 6) as xp,
            tc.tile_pool(name="yp", bufs=6) as yp,
            tc.tile_pool(name="ps", bufs=8, space="PSUM") as psp,
        ):
            # ---- emission order per engine is execution order ----
            # sync:   x DMAs (group 0 split per k-tile), tail y DMAs
            # scalar: W per-k DMAs, bias DMA, per-group y DMAs
            # tensor: warmup matmuls, then the real matmul stream
            # vector: per-subtile bias-add + PSUM->SBUF copy
            # gpsimd: one small memset for the warmup operands

            # W rides the (fast) sync ring ahead of the x stream; k-split
            # so the first matmul only waits on one 128 KB slice.
            wk = const.tile([P, KT, BOUT], w_dt)
            for k in range(KT):
                nc.sync.dma_start(wk[:, k, :], wT[k * P : (k + 1) * P, :])
            g0 = groups[0]
            x0k = []
            for k in range(KT):
                t = const.tile([P, g0], x_dt)
                nc.sync.dma_start(t[:], xT[k * P : (k + 1) * P, :g0])
                x0k.append(t)

            # scalar ring: bias, then all the y-out DMAs — its slow early
            # phase is absorbed by the deep yt pool.
            bt = const.tile([P, BOUT], F32)
            nc.scalar.dma_start(bt[:], bias[:])

            # PE warmup: fp16 matmuls with a cheap memset dependency keep
            # the PE busy until the HAM clock-gate flips to full speed
            # (~3.4us) and W/x0 have landed; 427ns each cold gives a fine
            # handoff granularity to the first real matmul.
            junk = const.tile([P, 640], mybir.dt.float16)
            nc.gpsimd.memset(junk[:], 0.0)
            warm_ps = psp.tile([P, BOUT], F32, tag="ps")
            for _ in range(8):
                nc.tensor.matmul(
                    warm_ps[:, :],
                    junk[:, :128],
                    junk[:, 128:640],
                    start=True,
                    stop=True,
                )

            row = 0
            for gi, gsz in enumerate(groups):
                nsub = gsz // P
                if gi == 0:
                    xs = None
                else:
                    xs = xp.tile([P, KT, gsz], x_dt, tag="xt")
                    nc.sync.dma_start(
                        xs[:],
                        xT[:, row : row + gsz].rearrange("(t p) b -> p t b", p=P),
                    )
                yt = yp.tile([P, nsub, BOUT], out_dt, tag="yt")
                for ms in range(nsub):
                    ps = psp.tile([P, BOUT], F32, tag="ps")
                    for k in range(KT):
                        lhsT = (
                            x0k[k][:, ms * P : (ms + 1) * P]
                            if gi == 0
                            else xs[:, k, ms * P : (ms + 1) * P]
                        )
                        nc.tensor.matmul(
                            ps[:],
                            lhsT,
                            wk[:, k, :],
                            start=(k == 0),
                            stop=(k == KT - 1),
                        )
                    nc.vector.tensor_add(out=yt[:, ms, :], in0=ps[:], in1=bt[:])
                # tail y DMAs ride the sync ring, idle once x-in is done
                eng = nc.sync if gi >= len(groups) - 2 else nc.scalar
                eng.dma_start(
                    y[row : row + gsz, :].rearrange("(s p) o -> p s o", p=P),
                    yt[:],
                )
                row += gsz
    nc.compile()
    _nc_cache[key] = nc
    return nc


LAST_RESULT = None


def kernel(x, W, b, trace=False, scheme=SCHEME, body=1536, trace_kwargs=None):
    global LAST_RESULT
    x = np.asarray(x, dtype=np.float32)
    W = np.asarray(W, dtype=np.float32)
    b = np.asarray(b, dtype=np.float32)

    _, x_np = _scheme_dtypes(scheme)
    nc = _build(scheme, body)
    in_maps = []
    for g in range(NCORES):
        xT_g = np.ascontiguousarray(x[:, g * BIN : (g + 1) * BIN].T.astype(x_np))
        wT_g = np.ascontiguousarray(W[g].T.astype(np.float16))
        bias_g = np.ascontiguousarray(np.broadcast_to(b[g][None, :], (P, BOUT)))
        in_maps.append({"xT": xT_g, "wT": wT_g, "bias": bias_g})

    kwargs = dict(trace_kwargs or {})
    res = run_bass_kernel_spmd(nc, in_maps, list(range(NCORES)), trace=trace, **kwargs)
    LAST_RESULT = res

    out = np.empty((BATCH, NB * BOUT), dtype=np.float32)
    for g in range(NCORES):
        out[:, g * BOUT : (g + 1) * BOUT] = res.results[g]["y"].astype(np.float32)
    return out
